# revision 3
# baseline (speedup 1.0000x reference)
"""Uniform cubic B-spline evaluation (KAN-style) on 8 Trainium2 NeuronCores.

Algorithm: telescoping Hermite evaluation. With tau = 31.5*(x+1) in [0, 63),
the spline y(tau) (piecewise cubic, C^2, unit knots) is

    y(tau) = y(63) + sum_{m=0}^{62} H_m(sigma_m),

where sigma_m = clamp(m + 1 - tau, 0, 1) is interval m's (descending)
saturating ramp and H_m a zero-constant cubic with H_m(1) = y(m) - y(m+1):
the saturated terms telescope the knot values. Every term is O(local spline
magnitude) — no cancellation, fp32-exact to ~1e-6 of scale — and there is no
data-dependent addressing (Trainium has no fast per-element gather; a 63-term
streaming sweep beats every gather/one-hot alternative by >5x).

Engine split, ONE instruction per interval on each engine:
  - ACT (scalar engine): w_m = relu(-31.5*x + (m - 30.5))   [free affine]
  - DVE (vector engine): one fused custom op
        t = min(w_m, 1);  chain += ((C2*t + C1)*t + C0)*t
    Per-interval cubic coefficients enter as instruction immediates,
    computed on host from `coeffs` in float64 (O(69) work).
  - Six accumulator chains hide the DVE's per-op pipeline DRAIN
    (measured: 8.56 us/op serial -> 3.97 us at 6 chains; odd chain counts
    serialize on SBUF banks); five adds merge them at the end.
"""

import numpy as np

from concourse import bacc, mybir
from concourse import dve_ops
from concourse.dve_spec import (
    Spec, Src0, Src1, C0, C1, C2, One, relu, minn, lower, _has_src1,
)
from concourse.dve_uop import DveOpSpec
from concourse.tile import TileContext
from concourse.bass_utils import run_bass_kernel_spmd

# ---------------------------------------------------------------- constants
N_POINTS = 4194304
N_CORES = 8
P = 128                       # SBUF partitions
F = N_POINTS // N_CORES // P  # 4096 free-dim elements per partition per core
M_INT = 63                    # spline intervals in tau-space
SCALE = 31.5                  # 1/h with h = 2/63; exact in fp32
N_CHAINS = 6                  # accumulator chains (DVE drain hiding; even counts pipeline, odd serialize)

_DT = mybir.dt.float32
_ALU = mybir.AluOpType


# ---------------------------------------------------- custom DVE op registry
def _register_op(name: str, spec: Spec) -> dve_ops.DveOp:
    """Register a custom DVE op at runtime (idempotent)."""
    for op in dve_ops.OPS:
        if op.name == name:
            return op
    row = dve_ops._CUSTOM_DVE_ROW_BASE + len(dve_ops.OPS)
    assert row < 0x20, "custom-DVE opcode rows exhausted"
    dve_ops._SUB_OPCODE_FOR_NAME[name] = row
    shas = {}
    for ver in ("v3", "v4"):
        try:
            s = DveOpSpec(name=name, opcode=row, uops=lower(spec, ver=ver),
                          rd1_en=_has_src1(spec))
            shas[ver] = s.sha(ver)
        except Exception:
            pass  # ver not encodable; TRN2 only needs v3
    op = dve_ops.DveOp(name, spec, subdim=False, uops_sha=shas)
    dve_ops.OPS.append(op)
    return op


# s = min(relu(Src0*C1 + C0), 1)
_BSP_CLAMP = _register_op(
    "BSP_CLAMP_ANT",
    Spec(body=minn(relu(Src0 * C1 + C0), One)),
)

# out = ((C2*Src0 + C1)*Src0 + C0)*Src0 + Src1
_BSP_HORNER = _register_op(
    "BSP_HORNER_ACC_ANT",
    Spec(body=((C2 * Src0 + C1) * Src0 + C0) * Src0 + Src1),
)

# t = min(Src0, 1); out = ((C2*t + C1)*t + C0)*t + Src1
_T = minn(Src0, One)
_BSP_HORNER_MIN = _register_op(
    "BSP_HORNER_MIN_ANT",
    Spec(body=((C2 * _T + C1) * _T + C0) * _T + Src1),
)


# ------------------------------------------------------------- host tables
def _hermite_tables(coeffs: np.ndarray):
    """Per-interval cubic coefficients (float64 on host).

    With sigma = clamp(m + 1 - tau, 0, 1) (descending ramp), interval m's
    contribution relative to its right edge is
        H_m(sigma) = g1*sigma + g2*sigma^2 + g3*sigma^3,
    obtained from the ascending-Hermite cubic G_m(s), s = 1 - sigma, by
    dropping the constant term (it telescopes into the y(63) init).
    """
    c = coeffs.astype(np.float64)
    i = np.arange(M_INT + 1)
    y_k = (c[i] + 4.0 * c[i + 1] + c[i + 2]) / 6.0
    dy_k = (c[i + 2] - c[i]) / 2.0
    d = y_k[1:] - y_k[:-1]
    b1 = dy_k[:-1]
    b2 = 3.0 * d - 2.0 * dy_k[:-1] - dy_k[1:]
    b3 = -2.0 * d + dy_k[:-1] + dy_k[1:]
    g1 = -(b1 + 2.0 * b2 + 3.0 * b3)
    g2 = b2 + 3.0 * b3
    g3 = -b3
    return y_k, d, g1, g2, g3


# ------------------------------------------------------------ module build
def _build_module(coeffs: np.ndarray, repeats: int = 1, n_chains: int = None):
    y_k, d, g1, g2, g3 = _hermite_tables(coeffs)
    if n_chains is None:
        n_chains = N_CHAINS
    acc_init = float(y_k[M_INT])  # y at tau=63; interval terms telescope down

    nc = bacc.Bacc("TRN2", target_bir_lowering=False, debug=False,
                   num_devices=N_CORES)
    x_ext = nc.dram_tensor("x", [P, F], _DT, kind="ExternalInput").ap()
    b_ext = nc.dram_tensor("b", [P, M_INT], _DT, kind="ExternalInput").ap()
    y_ext = nc.dram_tensor("y", [P, F], _DT, kind="ExternalOutput").ap()

    relu_fn = mybir.ActivationFunctionType.Relu

    with TileContext(nc) as tc:
        with tc.tile_pool(name="sbuf", bufs=1) as pool, \
             tc.tile_pool(name="ramp", bufs=5) as rpool:
            bt = pool.tile([P, M_INT], _DT)
            nc.sync.dma_start(out=bt[:], in_=b_ext[:])
            xt = pool.tile([P, F], _DT)
            accs = [pool.tile([P, F], _DT, tag=f"acc{j}", name=f"acc{j}")
                    for j in range(n_chains)]

            def _sweep():
                nc.sync.dma_start(out=xt[:], in_=x_ext[:])
                for j, a in enumerate(accs):
                    nc.gpsimd.memset(a[:], acc_init if j == 0 else 0.0)
                for m in range(M_INT):
                    w = rpool.tile([P, F], _DT, tag="w")
                    nc.scalar.activation(w[:], xt[:], relu_fn,
                                         bias=bt[:, m:m + 1], scale=-SCALE)
                    a = accs[m % n_chains]
                    nc.vector._custom_dve(
                        _BSP_HORNER_MIN, out=a[:], in0=w[:], in1=a[:],
                        s0=float(g1[m]), s1=float(g2[m]), imm2=float(g3[m]))
                # merge chains pairwise (independent adds pipeline on DVE)
                live = list(range(n_chains))
                while len(live) > 1:
                    nxt = []
                    for k in range(0, len(live) - 1, 2):
                        i0, i1 = live[k], live[k + 1]
                        nc.vector.tensor_add(out=accs[i0][:], in0=accs[i0][:],
                                             in1=accs[i1][:])
                        nxt.append(i0)
                    if len(live) % 2:
                        nxt.append(live[-1])
                    live = nxt
                nc.sync.dma_start(out=y_ext[:], in_=accs[0][:])

            if repeats == 1:
                _sweep()
            else:
                with tc.For_i(0, repeats, 1):
                    _sweep()

    nc.compile()
    return nc


def _bias_array() -> np.ndarray:
    cols = np.arange(M_INT, dtype=np.float32) - np.float32(30.5)
    return np.broadcast_to(cols[None, :], (P, M_INT)).copy()


_CACHE: dict[bytes, object] = {}


def kernel(x: np.ndarray, coeffs: np.ndarray, grid: np.ndarray) -> np.ndarray:
    x = np.asarray(x)
    coeffs = np.asarray(coeffs)
    key = coeffs.tobytes()
    nc = _CACHE.get(key)
    if nc is None:
        nc = _build_module(coeffs)
        _CACHE[key] = nc

    shards = x.astype(np.float32).reshape(N_CORES, P, F)
    bias = _bias_array()
    in_maps = [{"x": shards[i], "b": bias} for i in range(N_CORES)]
    res = run_bass_kernel_spmd(nc, in_maps, list(range(N_CORES)))
    out = np.empty((N_CORES, P, F), dtype=np.float32)
    for i in range(N_CORES):
        out[i] = res.results[i]["y"]
    return out.reshape(N_POINTS)


def bench_module(inputs, repeats: int):
    """(nc, in_maps) with the sweep wrapped in a hardware loop of `repeats`."""
    x = np.asarray(inputs["x"]).astype(np.float32)
    coeffs = np.asarray(inputs["coeffs"])
    nc = _build_module(coeffs, repeats=repeats)
    shards = x.reshape(N_CORES, P, F)
    bias = _bias_array()
    in_maps = [{"x": shards[i], "b": bias} for i in range(N_CORES)]
    return nc, in_maps

